# revision 34
# baseline (speedup 1.0000x reference)
"""Encoder self-attention (RMSNorm + fused QKV + qk-norm + SDPA + scaled o_proj
+ residual) on 8 NeuronCores, data-parallel over the batch dim N=8.

v3: head-pair-major software pipeline.
- QKV for one head pair via a step-sliced rhs (q|k|v 128-col blocks).
- qk-norm is scale-invariant => input-RMSNorm applied only to v.
- q/k PSUM evicted to bf16 staging; per-head sum-of-squares on Pool/DVE;
  norm factors batched per head pair as exp(-0.5*ln(ssq+hd*eps)) on ACT
  (one pinned activation table: ln/exp/square -> no table thrash).
- Transposed to [chan, tok] via one combined DMA-transpose per token tile.
- scores -> exp(bf16) -> AV with a ones column producing softmax
  denominators -> reciprocal_approx_fast -> bf16 K=1 outer-product
  broadcast matmul -> normalize.
- Emission is software-pipelined: QKV(hp+1) is emitted before attention(hp)
  so the PE never waits for fresh transposes.
"""

import numpy as np
import ml_dtypes
from contextlib import ExitStack

import concourse.bass as bass
import concourse.mybir as mybir
import concourse.tile as tile
from concourse import bacc
from concourse.bass import ts

F32 = mybir.dt.float32
BF16 = mybir.dt.bfloat16
AF = mybir.ActivationFunctionType

P = 128
D = 768
L = 1024
NH = 12
HD = 64
TQ = L // P      # 8 token tiles
KC = D // P      # 6 contraction chunks
NHP = NH // 2    # 6 head pairs
EPS = 1e-6
# act_info.json act_func_sets index of natural_log_exp_and_others
ACT_SET_LN_EXP = 6


def build_bass():
    nc = bacc.Bacc(None, target_bir_lowering=False)

    xT_d = nc.dram_tensor("xT", [D, L], BF16, kind="ExternalInput")
    wT_d = nc.dram_tensor("wT", [D, 3 * D], BF16, kind="ExternalInput")
    oT_d = nc.dram_tensor("oT", [D, D], BF16, kind="ExternalInput")
    xb_d = nc.dram_tensor("xb", [L, D], BF16, kind="ExternalInput")
    out_d = nc.dram_tensor("out", [L, D], F32, kind="ExternalOutput")

    with tile.TileContext(nc) as tc, ExitStack() as ctx:
        persist = ctx.enter_context(tc.tile_pool(name="persist", bufs=1))
        ets_pool = ctx.enter_context(tc.tile_pool(name="ets", bufs=2))
        scr = ctx.enter_context(tc.tile_pool(name="scr", bufs=2))
        qkvp = ctx.enter_context(tc.tile_pool(name="qkvp", bufs=2, space="PSUM"))
        scp = ctx.enter_context(tc.tile_pool(name="scp", bufs=2, space="PSUM"))
        avp = ctx.enter_context(tc.tile_pool(name="avp", bufs=2, space="PSUM"))

        # pin the ln/exp/square activation table once: every ACT func used
        # here is served by it, so no per-activation table reloads
        atl = mybir.InstLoadActFuncSet(
            name=nc.get_next_instruction_name(),
            ins=[],
            outs=[],
            act_func_set_id=ACT_SET_LN_EXP,
        )
        nc.scalar.add_instruction(atl)

        # ---- persistent SBUF ----
        xT_all = persist.tile([P, KC, L], BF16, tag="xT_all", name="xT_all")
        wT_all = persist.tile([P, KC, 18, P], BF16, tag="wT_all", name="wT_all")
        # qkT[:, 0] = qnT, qkT[:, 1] = knT  (chan-major, [chan, tok] blocks)
        qkT = persist.tile([P, 2, NHP, L], BF16, tag="qkT", name="qkT")
        v_sb = [
            persist.tile([P, NH, HD + 1], BF16, tag=f"v{j}", name=f"v{j}")
            for j in range(TQ)
        ]
        attnT = persist.tile([P, KC, L], BF16, tag="attnT", name="attnT")
        oT_all = persist.tile([P, KC, D], BF16, tag="oT_all", name="oT_all")
        xb_all = persist.tile([P, TQ, D], BF16, tag="xb_all", name="xb_all")
        rstd = persist.tile([P, TQ], F32, tag="rstd")
        ones_b = persist.tile([1, HD], BF16, tag="ones_b")
        nc.gpsimd.memset(ones_b[:], 1.0)
        eps_t = persist.tile([P, 1], F32, tag="eps_t")
        nc.gpsimd.memset(eps_t[:], EPS)
        eps64_t = persist.tile([P, 1], F32, tag="eps64_t")
        nc.gpsimd.memset(eps64_t[:], HD * EPS)

        # ---- input DMAs: weights split small for queue parallelism (SP
        # queue); xb/x/oT ride the ACT hwdge queue ----
        for d in range(KC):
            for h2 in range(2):
                nc.sync.dma_start(
                    out=xT_all[:, d, ts(h2, 512)], in_=xT_d[ts(d, P), ts(h2, 512)]
                )
            for g in range(3):
                nc.sync.dma_start(
                    out=wT_all[:, d, 6 * g : 6 * g + 6, :],
                    in_=wT_d[ts(d, P), ts(g, D)].rearrange("p (g c) -> p g c", c=P),
                )
        for i in range(TQ):
            nc.gpsimd.memset(v_sb[i][:, :, HD : HD + 1], 1.0)

        # ---- rstd from the bf16 copy of x, pipelined per token tile ----
        ssq_all = scr.tile([P, TQ], F32, tag="ssq_all", bufs=1)
        lnr = scr.tile([P, TQ], F32, tag="lnr", bufs=1)
        for i in range(TQ):
            nc.scalar.dma_start(out=xb_all[:, i, :], in_=xb_d[ts(i, P), :])
            sqb = scr.tile([P, D], BF16, tag="sqb")
            nc.vector.tensor_mul(sqb[:], xb_all[:, i, :], xb_all[:, i, :])
            nc.vector.tensor_reduce(
                ssq_all[:, i : i + 1],
                sqb[:],
                axis=mybir.AxisListType.X,
                op=mybir.AluOpType.add,
            )
            nc.scalar.activation(
                lnr[:, i : i + 1], ssq_all[:, i : i + 1], AF.Ln,
                scale=1.0 / D, bias=eps_t[:],
            )
            nc.scalar.activation(
                rstd[:, i : i + 1], lnr[:, i : i + 1], AF.Exp, scale=-0.5
            )
        nc.scalar.dma_start(
            out=oT_all[:], in_=oT_d[:].rearrange("(c p) e -> p c e", p=P)
        )

        # ================= interleaved main pipeline =================
        def emit_qkv_block(hp, i, tus, ssg_all):
            ps = qkvp.tile([P, 3, P], F32, tag="qkv")
            for d in range(KC):
                nc.tensor.matmul(
                    ps[:],
                    xT_all[:, d, ts(i, P)],
                    wT_all[:, d, hp::NHP, :],
                    start=(d == 0),
                    stop=(d == KC - 1),
                )
            tu = scr.tile([P, 4, HD], BF16, tag=f"tu{i}", bufs=1)
            nc.vector.tensor_copy(
                tu[:].rearrange("p h d -> p (h d)"),
                ps[:, 0:2, :].rearrange("p a c -> p (a c)"),
            )
            tus.append(tu)
            nc.vector.tensor_scalar_mul(
                v_sb[i][:, 2 * hp : 2 * hp + 2, 0:HD],
                ps[:, 2, :].rearrange("p (h d) -> p h d", d=HD),
                rstd[:, i : i + 1],
            )
            sqg = scr.tile([P, 4, HD], BF16, tag="sqg")
            nc.vector.tensor_mul(sqg[:], tu[:], tu[:])
            nc.vector.tensor_reduce(
                ssg_all[:, i, :],
                sqg[:],
                axis=mybir.AxisListType.X,
                op=mybir.AluOpType.add,
            )

        def emit_norm_chain(hp, tus, ssg_all):
            lng = scr.tile([P, TQ, 4], F32, tag="lng")
            nc.scalar.activation(
                lng[:].rearrange("p i h -> p (i h)"),
                ssg_all[:].rearrange("p i h -> p (i h)"),
                AF.Ln,
                bias=eps64_t[:],
            )
            r_all = scr.tile([P, TQ, 4, 1], F32, tag="r_all")
            nc.scalar.activation(
                r_all[:, :, :, 0].rearrange("p i h -> p (i h)"),
                lng[:].rearrange("p i h -> p (i h)"),
                AF.Exp,
                scale=-0.5,
            )
            nc.vector.tensor_scalar_mul(
                r_all[:, :, 2:4, 0], r_all[:, :, 2:4, 0], 8.0
            )
            for i in range(TQ):
                tn = scr.tile([P, 4, HD], BF16, tag="tn", bufs=3)
                nc.gpsimd.tensor_mul(
                    tn[:], tus[i][:], r_all[:, i, :, :].to_broadcast((P, 4, HD))
                )
                nc.sync.dma_start_transpose(
                    out=qkT[:, :, hp, ts(i, P)],
                    in_=tn[:].rearrange("p h d -> p (h d)"),
                )

        def emit_scores(hp, hh, jt, ets):
            off = HD * hh
            sc = scp.tile([P, L], F32, tag="sc")
            for ic in range(2):
                nc.tensor.matmul(
                    sc[:, ts(ic, 512)],
                    qkT[off : off + HD, 1, hp, ts(jt, P)],
                    qkT[off : off + HD, 0, hp, ts(ic, 512)],
                    start=True,
                    stop=True,
                    tile_position=(off, 0),
                )
            et = ets_pool.tile([P, L], BF16, tag=f"ets{jt}_{hh}")
            nc.scalar.activation(et[:], sc[:], AF.Exp)
            ets[(jt, hh)] = et

        def emit_av(hp, hh, ic, ets, rinv_b):
            h = 2 * hp + hh
            av = avp.tile([HD + 1, 512], F32, tag="av")
            for jt in range(TQ):
                nc.tensor.matmul(
                    av[:],
                    v_sb[jt][:, h, :],
                    ets[(jt, hh)][:, ts(ic, 512)],
                    start=(jt == 0),
                    stop=(jt == TQ - 1),
                )
            if ic == 0:
                nc.vector.tensor_copy(
                    attnT[HD * hh : HD * hh + HD, hp, ts(ic, 512)], av[0:HD, :]
                )
            else:
                nc.scalar.copy(
                    attnT[HD * hh : HD * hh + HD, hp, ts(ic, 512)], av[0:HD, :]
                )
            rl = scr.tile([1, 512], F32, tag="rl", bufs=1)
            rl2 = scr.tile([1, 512], F32, tag="rl2", bufs=1)
            if hh == 0:
                nc.scalar.copy(rl[:], av[HD : HD + 1, :])
            else:
                nc.vector.tensor_copy(rl[:], av[HD : HD + 1, :])
            nc.vector.reciprocal_approx_fast(out=rl2[:], in_=rl[:])
            if hh == 0:
                nc.vector.tensor_copy(rinv_b[0:1, hh, ts(ic, 512)], rl2[:])
            else:
                nc.scalar.copy(rinv_b[0:1, hh, ts(ic, 512)], rl2[:])

        def emit_norm_mult(hp, hh, ic, rinv_b):
            off = HD * hh
            bc = avp.tile([HD + 1, 512], F32, tag="av")
            nc.tensor.matmul(
                bc[0:HD, :],
                ones_b[:],
                rinv_b[0:1, hh, ts(ic, 512)],
                start=True,
                stop=True,
            )
            nc.vector.tensor_mul(
                attnT[off : off + HD, hp, ts(ic, 512)],
                attnT[off : off + HD, hp, ts(ic, 512)],
                bc[0:HD, :],
            )

        state = {}
        for hp in range(NHP):
            tus = []
            ssg_all = scr.tile([P, TQ, 4], F32, tag="ssg_all")
            prev = hp - 1
            for k in range(TQ):
                emit_qkv_block(hp, k, tus, ssg_all)
                if prev >= 0:
                    p_ets, p_rinv = state["ets"], state["rinv_b"]
                    hh, pair = divmod(k, 4)
                    emit_scores(prev, hh, 2 * pair, p_ets)
                    emit_scores(prev, hh, 2 * pair + 1, p_ets)
                    if k == 3:
                        emit_av(prev, 0, 0, p_ets, p_rinv)
                        emit_av(prev, 0, 1, p_ets, p_rinv)
                    if k == 7:
                        emit_av(prev, 1, 0, p_ets, p_rinv)
                        emit_av(prev, 1, 1, p_ets, p_rinv)
                        for hh2 in range(2):
                            for ic in range(2):
                                emit_norm_mult(prev, hh2, ic, p_rinv)
            emit_norm_chain(hp, tus, ssg_all)
            state = {
                "ets": {},
                "rinv_b": scr.tile([1, 2, L], BF16, tag="rinv_b", bufs=1, name="rinv_b"),
            }
            # pre-register ets dict for this hp (filled by emit_scores)

        # ---- last head pair's attention, fused with the o_proj tail ----
        last = NHP - 1
        l_ets, l_rinv = state["ets"], state["rinv_b"]
        for hh in range(2):
            for jt in range(TQ):
                emit_scores(last, hh, jt, l_ets)
            emit_av(last, hh, 0, l_ets, l_rinv)
            emit_av(last, hh, 1, l_ets, l_rinv)

        def emit_oproj(i):
            o = scp.tile([P, L], F32, tag="sc")
            for c in range(KC):
                nc.tensor.matmul(
                    o[:, 0:512],
                    attnT[:, c, ts(i, P)],
                    oT_all[:, c, 0:512],
                    start=(c == 0),
                    stop=(c == KC - 1),
                )
            for c in range(KC):
                nc.tensor.matmul(
                    o[:, 512:D],
                    attnT[:, c, ts(i, P)],
                    oT_all[:, c, 512:D],
                    start=(c == 0),
                    stop=(c == KC - 1),
                )
            pair, j = divmod(i, 2)
            if j == 0:
                state["osb"] = scr.tile([P, 2, D], F32, tag="osb", name="osb")
            osb = state["osb"]
            nc.vector.tensor_add(
                osb[:, j, 0:512], o[:, 0:512], xb_all[:, i, 0:512]
            )
            nc.vector.tensor_add(
                osb[:, j, 512:D], o[:, 512:D], xb_all[:, i, 512:D]
            )
            if j == 1:
                nc.sync.dma_start(
                    out=out_d[ts(pair, 2 * P), :].rearrange(
                        "(a p) d -> p a d", p=P
                    ),
                    in_=osb[:],
                )

        # normalize token half 0 of the last pair first, run o_proj on those
        # tiles while half 1 normalizes
        for hh in range(2):
            emit_norm_mult(last, hh, 0, l_rinv)
        for i in range(4):
            emit_oproj(i)
        for hh in range(2):
            emit_norm_mult(last, hh, 1, l_rinv)
        for i in range(4, TQ):
            emit_oproj(i)

    nc.compile()
    return nc


_NC = None


def _get_nc():
    global _NC
    if _NC is None:
        _NC = build_bass()
    return _NC


def make_in_maps(input_NHWD, qkv_weight, o_weight, o_scale):
    N = input_NHWD.shape[0]
    wT = np.ascontiguousarray(
        qkv_weight.reshape(3 * D, D).T.astype(np.float32)
    ).astype(ml_dtypes.bfloat16)
    oT = np.ascontiguousarray(
        (o_weight * o_scale[:, None]).T.astype(np.float32)
    ).astype(ml_dtypes.bfloat16)
    in_maps = []
    for i in range(N):
        xi = np.ascontiguousarray(input_NHWD[i].reshape(L, D).astype(np.float32))
        in_maps.append(
            {
                "xT": np.ascontiguousarray(xi.T).astype(ml_dtypes.bfloat16),
                "wT": wT,
                "oT": oT,
                "xb": xi.astype(ml_dtypes.bfloat16),
            }
        )
    return in_maps


def kernel(input_NHWD, qkv_weight, o_weight, o_scale):
    import time
    from concourse.bass_utils import run_bass_kernel_spmd

    input_NHWD = np.asarray(input_NHWD)
    N, H, W, _ = input_NHWD.shape
    nc = _get_nc()
    in_maps = make_in_maps(
        np.asarray(input_NHWD),
        np.asarray(qkv_weight),
        np.asarray(o_weight),
        np.asarray(o_scale),
    )
    last_err = None
    for attempt in range(3):
        try:
            res = run_bass_kernel_spmd(nc, in_maps, list(range(N)))
            out = np.stack([res.results[i]["out"] for i in range(N)], axis=0)
            return out.reshape(N, H, W, D).astype(np.float32)
        except Exception as e:  # transient device wedge: clear + retry
            last_err = e
            try:
                import jax

                jax.clear_caches()
                jax.clear_backends()
            except Exception:
                pass
            time.sleep(10)
    raise last_err


# revision 35
# speedup vs baseline: 1.0445x; 1.0445x over previous
"""Encoder self-attention (RMSNorm + fused QKV + qk-norm + SDPA + scaled o_proj
+ residual) on 8 NeuronCores, data-parallel over the batch dim N=8.

v3: head-pair-major software pipeline.
- QKV for one head pair via a step-sliced rhs (q|k|v 128-col blocks).
- qk-norm is scale-invariant => input-RMSNorm applied only to v.
- q/k PSUM evicted to bf16 staging; per-head sum-of-squares on Pool/DVE;
  norm factors batched per head pair as exp(-0.5*ln(ssq+hd*eps)) on ACT
  (one pinned activation table: ln/exp/square -> no table thrash).
- Transposed to [chan, tok] via one combined DMA-transpose per token tile.
- scores -> exp(bf16) -> AV with a ones column producing softmax
  denominators -> reciprocal_approx_fast -> bf16 K=1 outer-product
  broadcast matmul -> normalize.
- Emission is software-pipelined: QKV(hp+1) is emitted before attention(hp)
  so the PE never waits for fresh transposes.
"""

import numpy as np
import ml_dtypes
from contextlib import ExitStack

import concourse.bass as bass
import concourse.mybir as mybir
import concourse.tile as tile
from concourse import bacc
from concourse.bass import ts

F32 = mybir.dt.float32
BF16 = mybir.dt.bfloat16
AF = mybir.ActivationFunctionType

P = 128
D = 768
L = 1024
NH = 12
HD = 64
TQ = L // P      # 8 token tiles
KC = D // P      # 6 contraction chunks
NHP = NH // 2    # 6 head pairs
EPS = 1e-6
# act_info.json act_func_sets index of natural_log_exp_and_others
ACT_SET_LN_EXP = 6


def build_bass():
    nc = bacc.Bacc(None, target_bir_lowering=False)

    xT_d = nc.dram_tensor("xT", [D, L], BF16, kind="ExternalInput")
    wT_d = nc.dram_tensor("wT", [D, 3 * D], BF16, kind="ExternalInput")
    oT_d = nc.dram_tensor("oT", [D, D], BF16, kind="ExternalInput")
    xb_d = nc.dram_tensor("xb", [L, D], BF16, kind="ExternalInput")
    out_d = nc.dram_tensor("out", [L, D], F32, kind="ExternalOutput")

    with tile.TileContext(nc) as tc, ExitStack() as ctx:
        persist = ctx.enter_context(tc.tile_pool(name="persist", bufs=1))
        ets_pool = ctx.enter_context(tc.tile_pool(name="ets", bufs=2))
        scr = ctx.enter_context(tc.tile_pool(name="scr", bufs=2))
        qkvp = ctx.enter_context(tc.tile_pool(name="qkvp", bufs=2, space="PSUM"))
        scp = ctx.enter_context(tc.tile_pool(name="scp", bufs=2, space="PSUM"))
        avp = ctx.enter_context(tc.tile_pool(name="avp", bufs=2, space="PSUM"))

        # pin the ln/exp/square activation table once: every ACT func used
        # here is served by it, so no per-activation table reloads
        atl = mybir.InstLoadActFuncSet(
            name=nc.get_next_instruction_name(),
            ins=[],
            outs=[],
            act_func_set_id=ACT_SET_LN_EXP,
        )
        nc.scalar.add_instruction(atl)

        # ---- persistent SBUF ----
        xT_all = persist.tile([P, KC, L], BF16, tag="xT_all", name="xT_all")
        wT_all = persist.tile([P, KC, NHP, 384], BF16, tag="wT_all", name="wT_all")
        # qkT[:, 0] = qnT, qkT[:, 1] = knT  (chan-major, [chan, tok] blocks)
        qkT = persist.tile([P, 2, NHP, L], BF16, tag="qkT", name="qkT")
        v_sb = [
            persist.tile([P, NH, HD + 1], BF16, tag=f"v{j}", name=f"v{j}")
            for j in range(TQ)
        ]
        attnT = persist.tile([P, KC, L], BF16, tag="attnT", name="attnT")
        oT_all = persist.tile([P, KC, D], BF16, tag="oT_all", name="oT_all")
        xb_all = persist.tile([P, TQ, D], BF16, tag="xb_all", name="xb_all")
        rstd = persist.tile([P, TQ], F32, tag="rstd")
        ones_b = persist.tile([1, HD], BF16, tag="ones_b")
        nc.gpsimd.memset(ones_b[:], 1.0)
        eps_t = persist.tile([P, 1], F32, tag="eps_t")
        nc.gpsimd.memset(eps_t[:], EPS)
        eps64_t = persist.tile([P, 1], F32, tag="eps64_t")
        nc.gpsimd.memset(eps64_t[:], HD * EPS)

        # ---- input DMAs: weights split small for queue parallelism (SP
        # queue); xb/x/oT ride the ACT hwdge queue ----
        for d in range(KC):
            for h2 in range(2):
                nc.sync.dma_start(
                    out=xT_all[:, d, ts(h2, 512)], in_=xT_d[ts(d, P), ts(h2, 512)]
                )
        for hp in range(NHP):
            for d in range(KC):
                nc.sync.dma_start(
                    out=wT_all[:, d, hp, :], in_=wT_d[ts(d, P), ts(hp, 384)]
                )
        for i in range(TQ):
            nc.gpsimd.memset(v_sb[i][:, :, HD : HD + 1], 1.0)

        # ---- rstd from the bf16 copy of x, pipelined per token tile ----
        ssq_all = scr.tile([P, TQ], F32, tag="ssq_all", bufs=1)
        lnr = scr.tile([P, TQ], F32, tag="lnr", bufs=1)
        for i in range(TQ):
            nc.scalar.dma_start(out=xb_all[:, i, :], in_=xb_d[ts(i, P), :])
            sqb = scr.tile([P, D], BF16, tag="sqb")
            nc.vector.tensor_mul(sqb[:], xb_all[:, i, :], xb_all[:, i, :])
            nc.vector.tensor_reduce(
                ssq_all[:, i : i + 1],
                sqb[:],
                axis=mybir.AxisListType.X,
                op=mybir.AluOpType.add,
            )
            nc.scalar.activation(
                lnr[:, i : i + 1], ssq_all[:, i : i + 1], AF.Ln,
                scale=1.0 / D, bias=eps_t[:],
            )
            nc.scalar.activation(
                rstd[:, i : i + 1], lnr[:, i : i + 1], AF.Exp, scale=-0.5
            )
        nc.scalar.dma_start(
            out=oT_all[:], in_=oT_d[:].rearrange("(c p) e -> p c e", p=P)
        )

        # ================= interleaved main pipeline =================
        def emit_qkv_block(hp, i, tus, ssg_all):
            ps = qkvp.tile([P, 3, P], F32, tag="qkv")
            for d in range(KC):
                nc.tensor.matmul(
                    ps[:],
                    xT_all[:, d, ts(i, P)],
                    wT_all[:, d, hp, :],
                    start=(d == 0),
                    stop=(d == KC - 1),
                )
            tu = scr.tile([P, 4, HD], BF16, tag=f"tu{i}", bufs=1)
            nc.vector.tensor_copy(
                tu[:].rearrange("p h d -> p (h d)"),
                ps[:, 0:2, :].rearrange("p a c -> p (a c)"),
            )
            tus.append(tu)
            nc.vector.tensor_scalar_mul(
                v_sb[i][:, 2 * hp : 2 * hp + 2, 0:HD],
                ps[:, 2, :].rearrange("p (h d) -> p h d", d=HD),
                rstd[:, i : i + 1],
            )
            sqg = scr.tile([P, 4, HD], BF16, tag="sqg")
            nc.vector.tensor_mul(sqg[:], tu[:], tu[:])
            nc.vector.tensor_reduce(
                ssg_all[:, i, :],
                sqg[:],
                axis=mybir.AxisListType.X,
                op=mybir.AluOpType.add,
            )

        def emit_norm_chain(hp, tus, ssg_all, half, r_all):
            i0 = 4 * half
            nc.scalar.activation(
                ssg_all[:, i0 : i0 + 4, :].rearrange("p i h -> p (i h)"),
                ssg_all[:, i0 : i0 + 4, :].rearrange("p i h -> p (i h)"),
                AF.Ln,
                bias=eps64_t[:],
            )
            nc.scalar.activation(
                r_all[:, i0 : i0 + 4, :, 0].rearrange("p i h -> p (i h)"),
                ssg_all[:, i0 : i0 + 4, :].rearrange("p i h -> p (i h)"),
                AF.Exp,
                scale=-0.5,
            )
            nc.vector.tensor_scalar_mul(
                r_all[:, i0 : i0 + 4, 2:4, 0], r_all[:, i0 : i0 + 4, 2:4, 0], 8.0
            )
            for i in range(i0, i0 + 4):
                tn = scr.tile([P, 4, HD], BF16, tag="tn", bufs=3)
                nc.gpsimd.tensor_mul(
                    tn[:], tus[i][:], r_all[:, i, :, :].to_broadcast((P, 4, HD))
                )
                nc.sync.dma_start_transpose(
                    out=qkT[:, :, hp, ts(i, P)],
                    in_=tn[:].rearrange("p h d -> p (h d)"),
                )

        def emit_scores(hp, hh, jt, ets):
            off = HD * hh
            sc = scp.tile([P, L], F32, tag="sc")
            for ic in range(2):
                nc.tensor.matmul(
                    sc[:, ts(ic, 512)],
                    qkT[off : off + HD, 1, hp, ts(jt, P)],
                    qkT[off : off + HD, 0, hp, ts(ic, 512)],
                    start=True,
                    stop=True,
                    tile_position=(off, 0),
                )
            et = ets_pool.tile([P, L], BF16, tag=f"ets{jt}_{hh}")
            nc.scalar.activation(et[:], sc[:], AF.Exp)
            ets[(jt, hh)] = et

        def emit_av(hp, hh, ic, ets, rinv_b):
            h = 2 * hp + hh
            av = avp.tile([HD + 1, 512], F32, tag="av")
            for jt in range(TQ):
                nc.tensor.matmul(
                    av[:],
                    v_sb[jt][:, h, :],
                    ets[(jt, hh)][:, ts(ic, 512)],
                    start=(jt == 0),
                    stop=(jt == TQ - 1),
                )
            if ic == 0:
                nc.vector.tensor_copy(
                    attnT[HD * hh : HD * hh + HD, hp, ts(ic, 512)], av[0:HD, :]
                )
            else:
                nc.scalar.copy(
                    attnT[HD * hh : HD * hh + HD, hp, ts(ic, 512)], av[0:HD, :]
                )
            rl = scr.tile([1, 512], F32, tag="rl", bufs=1)
            rl2 = scr.tile([1, 512], F32, tag="rl2", bufs=1)
            if hh == 0:
                nc.scalar.copy(rl[:], av[HD : HD + 1, :])
            else:
                nc.vector.tensor_copy(rl[:], av[HD : HD + 1, :])
            nc.vector.reciprocal_approx_fast(out=rl2[:], in_=rl[:])
            if hh == 0:
                nc.vector.tensor_copy(rinv_b[0:1, hh, ts(ic, 512)], rl2[:])
            else:
                nc.scalar.copy(rinv_b[0:1, hh, ts(ic, 512)], rl2[:])

        def emit_norm_mult(hp, hh, ic, rinv_b):
            off = HD * hh
            bc = avp.tile([HD + 1, 512], F32, tag="av")
            nc.tensor.matmul(
                bc[0:HD, :],
                ones_b[:],
                rinv_b[0:1, hh, ts(ic, 512)],
                start=True,
                stop=True,
            )
            nc.vector.tensor_mul(
                attnT[off : off + HD, hp, ts(ic, 512)],
                attnT[off : off + HD, hp, ts(ic, 512)],
                bc[0:HD, :],
            )

        state = {}
        for hp in range(NHP):
            tus = []
            ssg_all = scr.tile([P, TQ, 4], F32, tag="ssg_all")
            r_all = scr.tile([P, TQ, 4, 1], F32, tag="r_all")
            prev = hp - 1
            for k in range(TQ):
                emit_qkv_block(hp, k, tus, ssg_all)
                if prev >= 0:
                    p_ets, p_rinv = state["ets"], state["rinv_b"]
                    hh, pair = divmod(k, 4)
                    emit_scores(prev, hh, 2 * pair, p_ets)
                    emit_scores(prev, hh, 2 * pair + 1, p_ets)
                    if k == 5:
                        emit_av(prev, 0, 0, p_ets, p_rinv)
                        emit_av(prev, 0, 1, p_ets, p_rinv)
                    if k == 7:
                        emit_av(prev, 1, 0, p_ets, p_rinv)
                        emit_av(prev, 1, 1, p_ets, p_rinv)
                        for hh2 in range(2):
                            for ic in range(2):
                                emit_norm_mult(prev, hh2, ic, p_rinv)
                if k == 3:
                    emit_norm_chain(hp, tus, ssg_all, 0, r_all)
                if k == 7:
                    emit_norm_chain(hp, tus, ssg_all, 1, r_all)
            state = {
                "ets": {},
                "rinv_b": scr.tile([1, 2, L], BF16, tag="rinv_b", bufs=1, name="rinv_b"),
            }

        # ---- last head pair's attention, fused with the o_proj tail ----
        last = NHP - 1
        l_ets, l_rinv = state["ets"], state["rinv_b"]
        for hh in range(2):
            for jt in range(TQ):
                emit_scores(last, hh, jt, l_ets)
            emit_av(last, hh, 0, l_ets, l_rinv)
            emit_av(last, hh, 1, l_ets, l_rinv)

        def emit_oproj(i):
            o = scp.tile([P, L], F32, tag="sc")
            for c in range(KC):
                nc.tensor.matmul(
                    o[:, 0:512],
                    attnT[:, c, ts(i, P)],
                    oT_all[:, c, 0:512],
                    start=(c == 0),
                    stop=(c == KC - 1),
                )
            for c in range(KC):
                nc.tensor.matmul(
                    o[:, 512:D],
                    attnT[:, c, ts(i, P)],
                    oT_all[:, c, 512:D],
                    start=(c == 0),
                    stop=(c == KC - 1),
                )
            pair, j = divmod(i, 2)
            if j == 0:
                state["osb"] = scr.tile([P, 2, D], F32, tag="osb", name="osb")
            osb = state["osb"]
            nc.vector.tensor_add(
                osb[:, j, 0:512], o[:, 0:512], xb_all[:, i, 0:512]
            )
            nc.vector.tensor_add(
                osb[:, j, 512:D], o[:, 512:D], xb_all[:, i, 512:D]
            )
            if j == 1:
                nc.sync.dma_start(
                    out=out_d[ts(pair, 2 * P), :].rearrange(
                        "(a p) d -> p a d", p=P
                    ),
                    in_=osb[:],
                )

        # normalize token half 0 of the last pair first, run o_proj on those
        # tiles while half 1 normalizes
        for hh in range(2):
            emit_norm_mult(last, hh, 0, l_rinv)
        for i in range(4):
            emit_oproj(i)
        for hh in range(2):
            emit_norm_mult(last, hh, 1, l_rinv)
        for i in range(4, TQ):
            emit_oproj(i)

    nc.compile()
    return nc


_NC = None


def _get_nc():
    global _NC
    if _NC is None:
        _NC = build_bass()
    return _NC


def make_in_maps(input_NHWD, qkv_weight, o_weight, o_scale):
    N = input_NHWD.shape[0]
    wT0 = qkv_weight.reshape(3 * D, D).T.astype(np.float32)
    # head-pair-major column groups: [q_hp | k_hp | v_hp] x 6
    wT = np.ascontiguousarray(
        np.concatenate(
            [
                np.concatenate(
                    [wT0[:, a * D + hp * P : a * D + (hp + 1) * P] for a in range(3)],
                    axis=1,
                )
                for hp in range(NHP)
            ],
            axis=1,
        )
    ).astype(ml_dtypes.bfloat16)
    oT = np.ascontiguousarray(
        (o_weight * o_scale[:, None]).T.astype(np.float32)
    ).astype(ml_dtypes.bfloat16)
    in_maps = []
    for i in range(N):
        xi = np.ascontiguousarray(input_NHWD[i].reshape(L, D).astype(np.float32))
        in_maps.append(
            {
                "xT": np.ascontiguousarray(xi.T).astype(ml_dtypes.bfloat16),
                "wT": wT,
                "oT": oT,
                "xb": xi.astype(ml_dtypes.bfloat16),
            }
        )
    return in_maps


def kernel(input_NHWD, qkv_weight, o_weight, o_scale):
    import time
    from concourse.bass_utils import run_bass_kernel_spmd

    input_NHWD = np.asarray(input_NHWD)
    N, H, W, _ = input_NHWD.shape
    nc = _get_nc()
    in_maps = make_in_maps(
        np.asarray(input_NHWD),
        np.asarray(qkv_weight),
        np.asarray(o_weight),
        np.asarray(o_scale),
    )
    last_err = None
    for attempt in range(3):
        try:
            res = run_bass_kernel_spmd(nc, in_maps, list(range(N)))
            out = np.stack([res.results[i]["out"] for i in range(N)], axis=0)
            return out.reshape(N, H, W, D).astype(np.float32)
        except Exception as e:  # transient device wedge: clear + retry
            last_err = e
            try:
                import jax

                jax.clear_caches()
                jax.clear_backends()
            except Exception:
                pass
            time.sleep(10)
    raise last_err
